# revision 41
# baseline (speedup 1.0000x reference)
"""Causal single-head attention (B=4, S=2048, D=1024) on 8 TRN2 NeuronCores.

Sharding: core c -> (batch b = c//2, half h = c%2). Every core runs the SAME
program: its 1024 query rows are 8 slots of 128 rows; slot s holds global
rows [256*s + 128*h, 256*s + 128*h + 128) of batch b, whose padded causal
key-length is 256*(s+1).

All matmuls run as fp8(e4m3) DoubleRow (2 contraction k-tiles per
instruction, 0.5 cycles/row of output) with hi/lo error compensation tuned
per path against the 2e-2 error budget:
  - V projection: 3-term  xh@wh + xh@wl + xl@wh   (~0.13% error; V errors
    pass straight into the output)
  - K/Q projections: 2-term  xh@wh + xl@wh = x@wh (w-quant error only; score
    errors are softmax-damped ~5x before reaching the output)
  - scores: 1-term on fp8(K^T), fp8(Q^T)
  - attn @ V: 3-term  ah@vh + ah@vl + al@vh
X^T and W (pre-scaled by 32) are split hi/lo on the host and shipped fp8, so
the kernel needs no PE transposes for inputs. Scores are computed transposed
(S^T[k, q] with keys on partitions) so the attention weights can be consumed
directly as DoubleRow stationaries by attn @ V -- no attention transposes
either. exp() output is split hi/lo on device (ACT exp -> Pool hi-copy ->
DVE subtract). The softmax denominator comes from an extra ones(=32)-column
DoubleRow matmul and cancels the 32x V scale exactly. The causal mask is one
128x128 f32 add per key-chunk (two host mask tiles, selected by parity).
Both weight applications are moved to the sharded query side by
associativity, deduplicating the per-pair projections with no communication:
  - V path:  a@(Xv@Wv) = (a@Xv)@Wv.  M^T = (a@Xv)^T is built per slot with
    keys contracting on partitions (Xv ships untransposed), stored hi/lo at
    M/4, then (M/4)@(32*Wv); the ones(=8) denominator cancels all scales.
  - K path:  scores^T = Xk@G^T with G^T = (32*Wk^T)@qt/32 built once over the
    core's own 1024 q columns (6.8us) instead of projecting K over all 2048
    keys (27us). G is stored hi/lo so the 2-term score matmul stays at the
    same error as the old fp8(K)-store path; Xk ships raw fp8 (hi only).
Score chunk pairs emit their narrow masked pieces (the ones m_t blocks on)
first; m_w lags m_t by two slots so M^T stores always land in time.
Measured: TimelineSim 101.7us/core, rel_rms 1.52e-2 (gate 2e-2) on the
8-core run.
"""

import numpy as np

import concourse.bacc as bacc
import concourse.mybir as mybir
import concourse.tile as tile
from concourse import bass_utils

B, S, D = 4, 2048, 1024
P = 128
DCP = 4              # pairs of 128-deep contraction tiles (d dim)
ECP = 4              # pairs of 128-wide e tiles
NSLOT = 8            # q tiles per core
NQ = NSLOT * P       # 1024 q rows per core
NKC = S // P         # 16 key chunks
WSCALE = 32.0        # host pre-scale on all three weights
SCALE_EFF = 1.0 / (WSCALE * float(np.sqrt(np.float32(S))))
NEG = -1.0e9

F32 = mybir.dt.float32
FP8 = mybir.dt.float8e4
BF16 = mybir.dt.bfloat16
DR = mybir.MatmulPerfMode.DoubleRow


def build_attention_nc():
    nc = bacc.Bacc("TRN2", target_bir_lowering=False)

    xk_h = nc.dram_tensor("xk_h", [P, DCP, 2, S], FP8, kind="ExternalInput")
    wk_t = nc.dram_tensor("wk_t", [P, DCP, 2, D], FP8, kind="ExternalInput")
    xv_h = nc.dram_tensor("xv_h", [P, NKC, D], FP8, kind="ExternalInput")
    xv_l = nc.dram_tensor("xv_l", [P, NKC, D], FP8, kind="ExternalInput")
    xq_h = nc.dram_tensor("xq_h", [P, DCP, 2, NQ], FP8, kind="ExternalInput")
    w_in = {}
    for t in ("v", "q"):
        comps = ("h", "l") if t == "v" else ("h",)
        for c in comps:
            w_in[t, c] = nc.dram_tensor(
                f"w{t}_{c}", [P, DCP, 2, D], FP8, kind="ExternalInput"
            )
    mask_a = nc.dram_tensor("mask_a", [P, P], F32, kind="ExternalInput")
    mask_b = nc.dram_tensor("mask_b", [P, P], F32, kind="ExternalInput")
    ones_in = nc.dram_tensor("ones32", [P, 2, 1], FP8, kind="ExternalInput")
    out = nc.dram_tensor("out", [NQ, D], BF16, kind="ExternalOutput")

    with tile.TileContext(nc) as tc:
        with (
            tc.tile_pool(name="res", bufs=1) as res,
            tc.tile_pool(name="wp", bufs=2) as wp,
            tc.tile_pool(name="xs", bufs=2) as xs,
            tc.tile_pool(name="tmpp", bufs=3) as tmpp,
            tc.tile_pool(name="outp", bufs=2) as outp,
            tc.tile_pool(name="recp", bufs=3) as recp,
        ):
            # K path by associativity: Xk stays raw (hi only), G = Q~ @ Wk^T
            xkr = res.tile([P, DCP, 2, S], FP8, tag="xkr", name="xkr")
            gt = {c: res.tile([P, DCP, 2, NQ], FP8, tag=f"gt{c}", name=f"gt{c}") for c in "hl"}
            xvr = {c: res.tile([P, NKC, D], FP8, tag=f"xvr{c}", name=f"xvr{c}") for c in "hl"}
            mt = {c: res.tile([P, 2 * DCP, NQ], FP8, tag=f"mt{c}", name=f"mt{c}") for c in "hl"}
            qt_h = res.tile([P, ECP, 2, NQ], FP8, tag="qth", name="qt_h")
            at = {c: res.tile([P, NKC, NQ], FP8, tag=f"at{c}", name=f"at{c}") for c in "hl"}
            ma_sb = res.tile([P, P], F32, tag="maska")
            mb_sb = res.tile([P, P], F32, tag="maskb")
            ones_sb = res.tile([P, 2, 1], FP8, tag="ones")

            def proj_psum(ps, stats, movs, stat_slice, mov_slice, terms):
                """len(terms) x 4 DR matmuls accumulating one psum group."""
                total = len(terms) * DCP
                n = 0
                for cs, cm in terms:
                    for dcp in range(DCP):
                        nc.tensor.matmul(
                            ps,
                            stat_slice(stats[cs], dcp),
                            mov_slice(movs[cm], dcp),
                            start=(n == 0),
                            stop=(n == total - 1),
                            perf_mode=DR,
                        )
                        n += 1

            def store_hilo(ps, hi_slice, lo_slice, hi_engine=None):
                eng = hi_engine or nc.scalar
                if eng is nc.scalar:
                    nc.scalar.copy(hi_slice, ps)
                else:
                    eng.tensor_copy(hi_slice, ps)
                nc.vector.tensor_tensor(
                    out=lo_slice, in0=ps, in1=hi_slice,
                    op=mybir.AluOpType.subtract,
                )

            # ================= K / V / Q projection =================
            with (
                tc.tile_pool(name="pp", bufs=6, space="PSUM") as pp,
                tc.tile_pool(name="pout", bufs=2, space="PSUM") as pout,
            ):
                # ---- Q first: qt[e, q] resident (2-term) ----
                wq = {"h": wp.tile([P, DCP, 2, D], FP8, tag="wh", name="wqh")}
                xq = {"h": xs.tile([P, DCP, 2, NQ], FP8, tag="xqh", bufs=1, name="xqh")}
                # critical path: one FIFO queue, dcp-split so PE starts early
                nc.sync.dma_start(wq["h"][:, 0:2, :, :], w_in["q", "h"][:, 0:2, :, :])
                nc.sync.dma_start(xq["h"][:, :, :, 0:512], xq_h[:, :, :, 0:512])
                nc.sync.dma_start(wq["h"][:, 2:4, :, :], w_in["q", "h"][:, 2:4, :, :])
                nc.sync.dma_start(xq["h"][:, :, :, 512:NQ], xq_h[:, :, :, 512:NQ])
                for qb in range(2):
                    for ec in range(8):
                        ps = pp.tile([P, 512], F32, tag="ps")
                        proj_psum(
                            ps, wq, xq,
                            lambda w, dcp, ec=ec: w[:, dcp, :, ec * P : (ec + 1) * P],
                            lambda x, dcp, qb=qb: x[:, dcp, :, qb * 512 : (qb + 1) * 512],
                            terms=(("h", "h"),),
                        )
                        nc.scalar.copy(
                            qt_h[:, ec // 2, ec % 2, qb * 512 : (qb + 1) * 512], ps
                        )

                # ---- G^T[d, q] = (32Wk^T) @ qt / 32, hi/lo (1-term matmul) ----
                wkt = {"h": wp.tile([P, DCP, 2, D], FP8, tag="wh", name="wkth")}
                nc.sync.dma_start(wkt["h"][:, 0:2, :, :], wk_t[:, 0:2, :, :])
                nc.sync.dma_start(wkt["h"][:, 2:4, :, :], wk_t[:, 2:4, :, :])
                nc.sync.dma_start(xkr[:, :, :, 0:1024], xk_h[:, :, :, 0:1024])
                nc.sync.dma_start(xkr[:, :, :, 1024:S], xk_h[:, :, :, 1024:S])
                for dc in range(8):
                    for qb in range(2):
                        ps = pp.tile([P, 512], F32, tag="ps")
                        for ecp in range(ECP):
                            nc.tensor.matmul(
                                ps,
                                wkt["h"][:, ecp, :, dc * P : (dc + 1) * P],
                                qt_h[:, ecp, :, qb * 512 : (qb + 1) * 512],
                                start=(ecp == 0),
                                stop=(ecp == ECP - 1),
                                perf_mode=DR,
                            )
                        hi = gt["h"][:, dc // 2, dc % 2, qb * 512 : (qb + 1) * 512]
                        lo = gt["l"][:, dc // 2, dc % 2, qb * 512 : (qb + 1) * 512]
                        nc.scalar.activation(
                            out=hi, in_=ps,
                            func=mybir.ActivationFunctionType.Copy, scale=1.0 / 32.0,
                        )
                        nc.vector.scalar_tensor_tensor(
                            out=lo, in0=ps, scalar=1.0 / 32.0, in1=hi,
                            op0=mybir.AluOpType.mult, op1=mybir.AluOpType.subtract,
                        )

                # ---- V path inputs: resident Xv (keys on partitions) + Wv ----
                xvr_src = {"h": xv_h, "l": xv_l}
                for c in "hl":
                    nc.sync.dma_start(xvr[c], xvr_src[c][:, :, :])
                wv = {c: wp.tile([P, DCP, 2, D], FP8, tag=f"w{c}", name=f"wv{c}") for c in "hl"}
                for c in "hl":
                    nc.sync.dma_start(wv[c], w_in["v", c][:, :, :, :])

                # ============ attention (same PSUM pools stay open) ============

                def score_piece(t, pq0, wp_, masked):
                    """One S^T piece of k-chunk t: psum, mask, exp, a hi/lo."""
                    ps = pp.tile([P, 512], F32, tag="ps", name="ps")
                    n = 0
                    for cg in "hl":
                        for dcp in range(DCP):
                            nc.tensor.matmul(
                                ps[:, :wp_],
                                xkr[:, dcp, :, t * P : (t + 1) * P],
                                gt[cg][:, dcp, :, pq0 : pq0 + wp_],
                                start=(n == 0),
                                stop=(n == 2 * DCP - 1),
                                perf_mode=DR,
                            )
                            n += 1
                    if masked:
                        nc.vector.tensor_tensor(
                            out=ps[:, 0:P], in0=ps[:, 0:P],
                            in1=(ma_sb if t % 2 == 0 else mb_sb),
                            op=mybir.AluOpType.add,
                        )
                    tmp = tmpp.tile([P, 512], F32, tag="tmp")
                    nc.scalar.activation(
                        out=tmp[:, :wp_], in_=ps[:, :wp_],
                        func=mybir.ActivationFunctionType.Exp,
                        scale=SCALE_EFF,
                    )
                    store_hilo(
                        tmp[:, :wp_],
                        at["h"][:, t, pq0 : pq0 + wp_],
                        at["l"][:, t, pq0 : pq0 + wp_],
                        hi_engine=nc.gpsimd,
                    )

                def score_pair(t0):
                    """Chunks (t0, t0+1): the narrow masked pieces (the ones
                    the next attn_v blocks on) first, then wides interleaved."""
                    q0 = P * (t0 // 2)
                    for t in (t0, t0 + 1):
                        score_piece(t, q0, P, masked=True)
                    wides = []
                    pq0 = q0 + P
                    while pq0 < NQ:
                        wp_ = min(512, NQ - pq0)
                        wides.append((pq0, wp_))
                        pq0 += wp_
                    for pq0, wp_ in wides:
                        for t in (t0, t0 + 1):
                            score_piece(t, pq0, wp_, masked=False)

                def m_t(s):
                    """M^T[d, q-slot s] = (a @ Xv)^T via 3-term DR, hi/lo/4 stored;
                    also the denominator (ones=8) and its reciprocal."""
                    npair = s + 1
                    ps_den_t = pp.tile([P, 512], F32, tag="ps", name="ps_den")
                    ps_den = ps_den_t[:, 0:1]
                    n = 0
                    for j in range(npair):
                        for c in "hl":
                            nc.tensor.matmul(
                                ps_den,
                                at[c][:, 2 * j : 2 * j + 2, s * P : (s + 1) * P],
                                ones_sb,
                                start=(n == 0),
                                stop=(n == 2 * npair - 1),
                                perf_mode=DR,
                            )
                            n += 1
                    rec = recp.tile([P, 1], F32, tag="rec")
                    nc.vector.reciprocal(rec, ps_den)
                    for g4 in range(2):
                        ps = pp.tile([P, 512], F32, tag="ps", name="ps_mt")
                        for ci in range(4):
                            dc = g4 * 4 + ci
                            n = 0
                            for j in range(npair):
                                for cx, ca in (("h", "h"), ("h", "l"), ("l", "h")):
                                    nc.tensor.matmul(
                                        ps[:, ci * P : (ci + 1) * P],
                                        xvr[cx][:, 2 * j : 2 * j + 2, dc * P : (dc + 1) * P],
                                        at[ca][:, 2 * j : 2 * j + 2, s * P : (s + 1) * P],
                                        start=(n == 0),
                                        stop=(n == 3 * npair - 1),
                                        perf_mode=DR,
                                    )
                                    n += 1
                        hi = mt["h"][:, g4 * 4 : (g4 + 1) * 4, s * P : (s + 1) * P]
                        lo = mt["l"][:, g4 * 4 : (g4 + 1) * 4, s * P : (s + 1) * P]
                        nc.scalar.activation(
                            out=hi, in_=ps,
                            func=mybir.ActivationFunctionType.Copy, scale=0.25,
                        )
                        nc.vector.scalar_tensor_tensor(
                            out=lo, in0=ps, scalar=0.25, in1=hi,
                            op0=mybir.AluOpType.mult, op1=mybir.AluOpType.subtract,
                        )
                    return rec

                def m_w(s, rec):
                    """out[q-slot s, :] = ((M/4) @ 32*Wv) / (8 * sum a)."""
                    out_sb = outp.tile([P, D], BF16, tag="outsb")
                    groups = [(0, 512), (512, 1024)]
                    if s == NSLOT - 1:
                        groups = [(0, 512), (512, 768), (768, 1024)]
                    for gi, (lo_e, hi_e) in enumerate(groups):
                        ps_o = pout.tile([P, 512], F32, tag="po")
                        w_ = hi_e - lo_e
                        n = 0
                        for cm, cw in (("h", "h"), ("h", "l"), ("l", "h")):
                            for dcp in range(DCP):
                                nc.tensor.matmul(
                                    ps_o[:, 0:w_],
                                    mt[cm][:, 2 * dcp : 2 * dcp + 2, s * P : (s + 1) * P],
                                    wv[cw][:, dcp, :, lo_e:hi_e],
                                    start=(n == 0),
                                    stop=(n == 3 * DCP - 1),
                                    perf_mode=DR,
                                )
                                n += 1
                        if gi % 2 == 0:
                            nc.scalar.activation(
                                out=out_sb[:, lo_e:hi_e],
                                in_=ps_o[:, 0:w_],
                                func=mybir.ActivationFunctionType.Copy,
                                scale=rec,
                            )
                        else:
                            nc.vector.tensor_scalar_mul(
                                out_sb[:, lo_e:hi_e], ps_o[:, 0:w_], rec
                            )
                        oq = nc.sync if gi % 2 == 0 else nc.scalar
                        oq.dma_start(
                            out[s * P : (s + 1) * P, lo_e:hi_e], out_sb[:, lo_e:hi_e]
                        )

                # pipeline: score chunks run one slot ahead of attn_v
                nc.scalar.dma_start(ma_sb, mask_a[:, :])
                nc.scalar.dma_start(mb_sb, mask_b[:, :])
                nc.scalar.dma_start(ones_sb, ones_in[:, :, :])
                # schedule: m_t(s) covers the exp latency of the score pair
                # just produced; m_w(s) runs after m_t(s+1) so M^T stores land
                score_pair(0)
                recs = {}
                recs[0] = m_t(0)
                for s in range(1, NSLOT):
                    score_pair(2 * s)
                    recs[s] = m_t(s)
                    if s >= 2:
                        m_w(s - 2, recs[s - 2])
                m_w(NSLOT - 2, recs[NSLOT - 2])
                m_w(NSLOT - 1, recs[NSLOT - 1])

    nc.compile()
    return nc


_NC_CACHE = None


def _get_nc():
    global _NC_CACHE
    if _NC_CACHE is None:
        _NC_CACHE = build_attention_nc()
    return _NC_CACHE


def _split8(a):
    import ml_dtypes

    f8 = ml_dtypes.float8_e4m3
    h = a.astype(np.float32).astype(f8)
    l = (a.astype(np.float32) - h.astype(np.float32)).astype(f8)
    return h, l


def _to_xt(a):
    """[rows, d] f32 component -> [128, DCP, 2, rows] fp8 (d on partitions)."""
    rows = a.shape[0]
    return np.ascontiguousarray(a.reshape(rows, DCP, 2, P).transpose(3, 1, 2, 0))


def _to_w(a):
    """[d, e] f32 component -> [128, DCP, 2, e] fp8 (d on partitions)."""
    return np.ascontiguousarray(a.reshape(DCP, 2, P, D).transpose(2, 0, 1, 3))


def _make_masks(h):
    i = np.arange(P)[:, None]
    j = np.arange(P)[None, :]
    if h == 0:
        a = np.where(j >= i, 0.0, NEG).astype(np.float32)  # k partition, q free
        b = np.full((P, P), NEG, dtype=np.float32)
    else:
        a = np.zeros((P, P), dtype=np.float32)
        b = np.where(j >= i, 0.0, NEG).astype(np.float32)
    return a, b


def kernel(
    inputs_for_keys,
    inputs_for_values,
    inputs_for_queries,
    weight_K,
    weight_V,
    weight_Q,
    trace=False,
):
    import ml_dtypes

    xk_full = np.asarray(inputs_for_keys, dtype=np.float32)
    xv_full = np.asarray(inputs_for_values, dtype=np.float32)
    xq_full = np.asarray(inputs_for_queries, dtype=np.float32)

    w_split = {}
    for name, w in (("v", weight_V), ("q", weight_Q)):
        wh, wl = _split8(np.asarray(w, dtype=np.float32) * WSCALE)
        w_split[name] = (_to_w(wh), _to_w(wl))
    # Wk ships transposed ([e,d], x32, hi only) for the G = Q~ @ Wk^T matmul
    wkt_h = _to_w(_split8(
        np.ascontiguousarray(np.asarray(weight_K, dtype=np.float32).T) * WSCALE
    )[0])
    # Q projection runs 2-term (w-hi only); only V ships its lo component

    xk_hi = [_to_xt(_split8(xk_full[b])[0]) for b in range(B)]

    def _to_kd(a):
        # [S, D] -> [128 (k%128), S//128, D]: keys on partitions
        return np.ascontiguousarray(
            a.reshape(NKC, P, D).transpose(1, 0, 2)
        )

    xv_split = [tuple(_to_kd(c) for c in _split8(xv_full[b])) for b in range(B)]

    ones32 = np.full((P, 2, 1), 8.0, dtype=ml_dtypes.float8_e4m3)
    masks = [_make_masks(0), _make_masks(1)]

    in_maps = []
    for c in range(2 * B):
        b, h = c // 2, c % 2
        rows = np.concatenate(
            [
                xq_full[b, 256 * s + P * h : 256 * s + P * h + P, :]
                for s in range(NSLOT)
            ],
            axis=0,
        )
        qh = _split8(rows)[0]
        ma, mb = masks[h]
        in_maps.append(
            {
                "xk_h": xk_hi[b], "wk_t": wkt_h,
                "xv_h": xv_split[b][0], "xv_l": xv_split[b][1],
                "xq_h": _to_xt(qh),
                "wv_h": w_split["v"][0], "wv_l": w_split["v"][1],
                "wq_h": w_split["q"][0],
                "mask_a": ma, "mask_b": mb,
                "ones32": ones32,
            }
        )

    nc = _get_nc()
    res = bass_utils.run_bass_kernel_spmd(
        nc, in_maps, core_ids=list(range(2 * B)), trace=trace
    )

    out = np.empty((B, S, D), dtype=np.float32)
    for c in range(2 * B):
        b, h = c // 2, c % 2
        o = np.asarray(res.results[c]["out"], dtype=np.float32)
        for s in range(NSLOT):
            out[b, 256 * s + P * h : 256 * s + P * h + P, :] = o[
                s * P : (s + 1) * P, :
            ]

    if trace:
        return out, res
    return out
